# revision 10
# baseline (speedup 1.0000x reference)
"""Trainium2 Bass kernel for NeuralCTLSTM cell (B=65536, H=256, 7 gates).

Data-parallel over 8 NeuronCores (8192 batch rows each). Per core:
  gates = h @ W_g^T + b_g  (7 gates, fp32r matmuls, K=256 in 2 chunks)
  5 sigmoid gates, z=tanh, decay=softplus (via ln(1+exp)), e=exp(-decay*dt)
  c_after = cbar + (c-cbar)*e ; outputs o_g, h_new, c_new, cbar_new, decay.

Processed in 64 row-blocks of 128 (partition dim = batch rows). Blocks are
grouped in supergroups of 16 for ACT table-set phasing:
  phase A (natural_log_exp set): transpose h (PE), d-gate matmul, u=exp,
    decay=ln(1+u), e=exp(-dt*decay)
  phase B (sigmoid set): 6-gate matmuls, sigmoid/tanh, DVE elementwise, DMA out.
"""

import sys

sys.path.insert(0, "/opt/trn_rl_repo")

from contextlib import ExitStack

import numpy as np

NCORES = 8
B, H, G = 65536, 256, 7
P = 128
BL = B // NCORES           # rows per core
NBLK = BL // P             # 64 row-blocks per core
SG = 16                    # blocks per supergroup (ACT table phase unit)
ST = 4                     # blocks per DVE/DMA batch

# our gate order [f, o, fbar, i, ibar, z, d] as indices into reference order
# reference: 0=input(i) 1=forget(f) 2=output(o) 3=ibar 4=fbar 5=z 6=decay
GATE_PERM = [1, 2, 4, 0, 3, 5, 6]

_NC = None
TRACE = False        # set by test harness to collect an ntff profile
LAST_RESULT = None   # BassKernelResults of the most recent run


def build_nc(nblk=NBLK, sg=SG, st=ST):
    from concourse import bacc, masks, mybir
    from concourse.tile import TileContext

    F32 = mybir.dt.float32
    F32R = mybir.dt.float32r
    BF16 = mybir.dt.bfloat16
    AF = mybir.ActivationFunctionType

    assert nblk % sg == 0 and sg % st == 0
    bl = nblk * P

    nc = bacc.Bacc("TRN2", target_bir_lowering=False, debug=False)

    h_d = nc.dram_tensor("h", [bl, H], F32, kind="ExternalInput")
    c_d = nc.dram_tensor("c", [bl, H], F32, kind="ExternalInput")
    cb_d = nc.dram_tensor("cbar", [bl, H], F32, kind="ExternalInput")
    ndt_d = nc.dram_tensor("negdt", [P, nblk], F32, kind="ExternalInput")
    wt_d = nc.dram_tensor("wt", [2, P, G * H], F32R, kind="ExternalInput")
    bia_d = nc.dram_tensor("bias", [1, G * H], BF16, kind="ExternalInput")

    og_d = nc.dram_tensor("og", [bl, H], F32, kind="ExternalOutput")
    hn_d = nc.dram_tensor("hn", [bl, H], F32, kind="ExternalOutput")
    cn_d = nc.dram_tensor("cn", [bl, H], F32, kind="ExternalOutput")
    cbn_d = nc.dram_tensor("cbn", [bl, H], F32, kind="ExternalOutput")
    dec_d = nc.dram_tensor("dec", [bl, H], F32, kind="ExternalOutput")

    h_r = h_d.rearrange("(n p) d -> n p d", p=P)
    c_r = c_d.rearrange("(n p) d -> n p d", p=P)
    cb_r = cb_d.rearrange("(n p) d -> n p d", p=P)
    og_r = og_d.rearrange("(n p) d -> n p d", p=P)
    hn_r = hn_d.rearrange("(n p) d -> n p d", p=P)
    cn_r = cn_d.rearrange("(n p) d -> n p d", p=P)
    cbn_r = cbn_d.rearrange("(n p) d -> n p d", p=P)
    dec_r = dec_d.rearrange("(n p) d -> n p d", p=P)

    def dsl(r, b0, n):  # dram slice of n row-blocks as [P, n, H]
        return r[b0 : b0 + n].rearrange("n p d -> p n d")

    with TileContext(nc) as tc, ExitStack() as ctx:
        pool = lambda name, bufs, **kw: ctx.enter_context(
            tc.tile_pool(name=name, bufs=bufs, **kw)
        )
        const = pool("const", 1)
        hp = pool("hp", 2)
        cp = pool("cp", 2)
        cbp = pool("cbp", 2)
        hTp = pool("hTp", 2)
        spp = pool("spp", 1)
        ep = pool("ep", 2)
        Ap = pool("Ap", 1)
        zp = pool("zp", 1)
        r2p = pool("r2p", 1)
        hhp = pool("hhp", 2)
        ptp = pool("ptp", 1, space="PSUM")
        pdp = pool("pdp", 1, space="PSUM")
        pgp = pool("pgp", 2, space="PSUM")

        wt_sb = const.tile([P, 2, G * H], F32R)
        nc.sync.dma_start(wt_sb[:], wt_d.rearrange("c k g -> k c g"))
        bia_sb = const.tile([1, G * H], BF16)
        nc.sync.dma_start(bia_sb[:], bia_d[:, :])
        ndt_sb = const.tile([P, nblk], F32)
        nc.sync.dma_start(ndt_sb[:], ndt_d[:, :])
        ones_sb = const.tile([1, P], BF16)
        nc.vector.memset(ones_sb[:], 1.0)
        ident = const.tile([P, P], F32)
        masks.make_identity(nc, ident[:])
        tokp = pool("tokp", 2)

        # ACT table-set phase tokens: zero-valued [P,1] tiles whose data deps
        # force all sigmoid-set ops of supergroup g after the exp/ln-set ops
        # of g (tokB), and the exp-set ops of g+1 after sigmoid phase of g
        # (tokA) — keeps table switches at 2 per supergroup.
        tokA = None  # gates u-ops of current supergroup
        r2_last = None  # last r2 tile of previous supergroup (tanh_c output)

        for g in range(nblk // sg):
            # ---------------- phase A: transpose + decay/e chain ----------------
            if r2_last is not None:
                tokA = tokp.tile([P, 1], F32)
                nc.vector.tensor_scalar_mul(tokA[:], r2_last[:, st - 1, H : H + 1], 0.0)
            hT = hTp.tile([P, sg, H], F32R)
            sp = spp.tile([P, sg, H], F32)
            e = ep.tile([P, sg, H], F32)
            for s in range(sg // st):
                b0 = g * sg + s * st
                h_t = hp.tile([P, st, H], F32)
                nc.sync.dma_start(h_t[:], dsl(h_r, b0, st))
                for j in range(st):
                    jj = s * st + j
                    tp = ptp.tile([P, H], F32)
                    nc.tensor.transpose(tp[:, 0:P], h_t[:, j, 0:P], ident[:])
                    nc.tensor.transpose(tp[:, P : 2 * P], h_t[:, j, P : 2 * P], ident[:])
                    nc.scalar.copy(hT[:, jj, :], tp[:])
                    dps = pdp.tile([P, H], F32)
                    nc.tensor.matmul(
                        dps[:], ones_sb[:], bia_sb[:, 6 * H : 7 * H],
                        start=True, stop=False,
                    )
                    for ck in range(2):
                        nc.tensor.matmul(
                            dps[:],
                            hT[:, jj, ck * P : (ck + 1) * P],
                            wt_sb[:, ck, 6 * H : 7 * H],
                            start=False, stop=(ck == 1),
                        )
                    if tokA is not None:  # u = exp(g_d)
                        nc.scalar.activation(sp[:, jj, :], dps[:], AF.Exp, bias=tokA[:])
                    else:
                        nc.scalar.activation(sp[:, jj, :], dps[:], AF.Exp)
            nc.scalar.activation(sp[:], sp[:], AF.Ln, bias=1.0)  # decay = ln(1+u)
            nc.sync.dma_start(dsl(dec_r, g * sg, sg), sp[:])
            for jj in range(sg):
                nc.scalar.activation(
                    e[:, jj, :], sp[:, jj, :], AF.Exp,
                    scale=ndt_sb[:, g * sg + jj : g * sg + jj + 1],
                )
            # ---------------- phase B: gates + elementwise + outputs ----------------
            tokB = tokp.tile([P, 1], F32)
            nc.vector.tensor_scalar_mul(tokB[:], e[:, sg - 1, 0:1], 0.0)
            for s in range(sg // st):
                b0 = g * sg + s * st
                c_t = cp.tile([P, st, H], F32)
                cb_t = cbp.tile([P, st, H], F32)
                nc.sync.dma_start(c_t[:], dsl(c_r, b0, st))
                nc.sync.dma_start(cb_t[:], dsl(cb_r, b0, st))
                A = Ap.tile([P, st, 5 * H], F32)
                z = zp.tile([P, st, H], F32)
                r2 = r2p.tile([P, st, 2 * H], F32)
                hh = hhp.tile([P, st, 2 * H], F32)
                for j in range(st):
                    jj = s * st + j
                    gps = pgp.tile([P, 6 * H], F32)
                    for grp in range(3):
                        lo = grp * 2 * H
                        nc.tensor.matmul(
                            gps[:, lo : lo + 2 * H], ones_sb[:],
                            bia_sb[:, lo : lo + 2 * H], start=True, stop=False,
                        )
                        for ck in range(2):
                            nc.tensor.matmul(
                                gps[:, lo : lo + 2 * H],
                                hT[:, jj, ck * P : (ck + 1) * P],
                                wt_sb[:, ck, lo : lo + 2 * H],
                                start=False, stop=(ck == 1),
                            )
                    nc.scalar.activation(
                        A[:, j, 0 : 5 * H], gps[:, 0 : 5 * H], AF.Sigmoid, bias=tokB[:]
                    )
                    nc.scalar.activation(
                        z[:, j, :], gps[:, 5 * H : 6 * H], AF.Tanh, bias=tokB[:]
                    )
                est = e[:, s * st : (s + 1) * st, :]
                nc.vector.tensor_sub(c_t[:], c_t[:], cb_t[:])
                nc.vector.tensor_mul(c_t[:], c_t[:], est)
                nc.vector.tensor_add(r2[:, :, 0:H], c_t[:], cb_t[:])  # c_after
                nc.scalar.activation(r2[:, :, H : 2 * H], r2[:, :, 0:H], AF.Tanh)
                nc.vector.tensor_mul(hh[:], A[:, :, 0 : 2 * H], r2[:])  # mf | h_new
                nc.vector.tensor_mul(A[:, :, 2 * H : 3 * H], A[:, :, 2 * H : 3 * H], cb_t[:])
                nc.vector.tensor_mul(A[:, :, 3 * H : 4 * H], A[:, :, 3 * H : 4 * H], z[:])
                nc.vector.tensor_mul(A[:, :, 4 * H : 5 * H], A[:, :, 4 * H : 5 * H], z[:])
                nc.sync.dma_start(dsl(og_r, b0, st), A[:, :, H : 2 * H])
                nc.vector.tensor_add(r2[:, :, 0:H], hh[:, :, 0:H], A[:, :, 3 * H : 4 * H])
                nc.vector.tensor_add(
                    r2[:, :, H : 2 * H], A[:, :, 2 * H : 3 * H], A[:, :, 4 * H : 5 * H]
                )
                nc.sync.dma_start(dsl(hn_r, b0, st), hh[:, :, H : 2 * H])
                nc.sync.dma_start(dsl(cn_r, b0, st), r2[:, :, 0:H])
                nc.sync.dma_start(dsl(cbn_r, b0, st), r2[:, :, H : 2 * H])
                r2_last = r2
    nc.compile()
    return nc


def prep_weights(W, b):
    """W [7,256,256] fp32, b [7,256] fp32 -> wt [2,128,1792] f32, bias [1,1792] bf16."""
    import ml_dtypes

    Wp = np.ascontiguousarray(W[GATE_PERM])          # [7,256,256] (g,o,h)
    wt = np.ascontiguousarray(np.transpose(Wp, (2, 0, 1)).reshape(H, G * H))
    wt = np.ascontiguousarray(wt.reshape(2, P, G * H)).astype(np.float32)
    bp = np.ascontiguousarray(b[GATE_PERM]).reshape(1, G * H)
    bia = bp.astype(ml_dtypes.bfloat16)
    return wt, bia


def kernel(inter_times, h_ti, c_ti, cbar, W, b):
    global _NC
    from concourse.bass_utils import run_bass_kernel_spmd

    if _NC is None:
        _NC = build_nc()
    nc = _NC

    inter_times = np.asarray(inter_times, dtype=np.float32)
    h_ti = np.ascontiguousarray(np.asarray(h_ti, dtype=np.float32))
    c_ti = np.ascontiguousarray(np.asarray(c_ti, dtype=np.float32))
    cbar = np.ascontiguousarray(np.asarray(cbar, dtype=np.float32))
    wt, bia = prep_weights(np.asarray(W, np.float32), np.asarray(b, np.float32))

    in_maps = []
    for i in range(NCORES):
        lo, hi = i * BL, (i + 1) * BL
        negdt = np.ascontiguousarray(
            -inter_times[lo:hi].reshape(NBLK, P).T
        ).astype(np.float32)
        in_maps.append(
            {
                "h": h_ti[lo:hi],
                "c": c_ti[lo:hi],
                "cbar": cbar[lo:hi],
                "negdt": negdt,
                "wt": wt,
                "bias": bia,
            }
        )

    global LAST_RESULT
    res = run_bass_kernel_spmd(nc, in_maps, core_ids=list(range(NCORES)), trace=TRACE)
    LAST_RESULT = res
    outs = res.results
    og = np.concatenate([outs[i]["og"] for i in range(NCORES)], axis=0)
    hn = np.concatenate([outs[i]["hn"] for i in range(NCORES)], axis=0)
    cn = np.concatenate([outs[i]["cn"] for i in range(NCORES)], axis=0)
    cbn = np.concatenate([outs[i]["cbn"] for i in range(NCORES)], axis=0)
    dec = np.concatenate([outs[i]["dec"] for i in range(NCORES)], axis=0)
    return (og, hn, cn, cbn, dec)


# revision 13
# speedup vs baseline: 2276.4463x; 2276.4463x over previous
"""Trainium2 Bass kernel for NeuralCTLSTM cell (B=65536, H=256, 7 gates).

Data-parallel over 8 NeuronCores (8192 batch rows each). Per core:
  gates = h @ W_g^T + b_g  (7 gates, fp32r matmuls, K=256 in 2 chunks)
  5 sigmoid gates, z=tanh, decay=softplus (via ln(1+exp)), e=exp(-decay*dt)
  c_after = cbar + (c-cbar)*e ; outputs o_g, h_new, c_new, cbar_new, decay.

Processed in 64 row-blocks of 128 (partition dim = batch rows). Blocks are
grouped in supergroups of 16 for ACT table-set phasing:
  phase A (natural_log_exp set): transpose h (PE), d-gate matmul, u=exp,
    decay=ln(1+u), e=exp(-dt*decay)
  phase B (sigmoid set): 6-gate matmuls, sigmoid/tanh, DVE elementwise, DMA out.
"""

import sys

sys.path.insert(0, "/opt/trn_rl_repo")

from contextlib import ExitStack

import numpy as np

NCORES = 8
B, H, G = 65536, 256, 7
P = 128
BL = B // NCORES           # rows per core
NBLK = BL // P             # 64 row-blocks per core
SG = 16                    # blocks per supergroup (ACT table phase unit)
ST = 4                     # blocks per DVE/DMA batch

# our gate order [f, o, fbar, i, ibar, z, d] as indices into reference order
# reference: 0=input(i) 1=forget(f) 2=output(o) 3=ibar 4=fbar 5=z 6=decay
GATE_PERM = [1, 2, 4, 0, 3, 5, 6]

_NC = None
TRACE = False        # set by test harness to collect an ntff profile
LAST_RESULT = None   # BassKernelResults of the most recent run


def build_nc(nblk=NBLK, sg=SG, st=ST):
    from concourse import bacc, masks, mybir
    from concourse.tile import TileContext

    F32 = mybir.dt.float32
    F32R = mybir.dt.float32r
    BF16 = mybir.dt.bfloat16
    AF = mybir.ActivationFunctionType

    assert nblk % sg == 0 and sg % st == 0
    bl = nblk * P

    nc = bacc.Bacc("TRN2", target_bir_lowering=False, debug=False)

    h_d = nc.dram_tensor("h", [bl, H], F32, kind="ExternalInput")
    c_d = nc.dram_tensor("c", [bl, H], F32, kind="ExternalInput")
    cb_d = nc.dram_tensor("cbar", [bl, H], F32, kind="ExternalInput")
    ndt_d = nc.dram_tensor("negdt", [P, nblk], F32, kind="ExternalInput")
    wt_d = nc.dram_tensor("wt", [2, P, G * H], F32R, kind="ExternalInput")
    bia_d = nc.dram_tensor("bias", [1, G * H], BF16, kind="ExternalInput")

    og_d = nc.dram_tensor("og", [bl, H], F32, kind="ExternalOutput")
    hn_d = nc.dram_tensor("hn", [bl, H], F32, kind="ExternalOutput")
    cn_d = nc.dram_tensor("cn", [bl, H], F32, kind="ExternalOutput")
    cbn_d = nc.dram_tensor("cbn", [bl, H], F32, kind="ExternalOutput")
    dec_d = nc.dram_tensor("dec", [bl, H], F32, kind="ExternalOutput")

    h_r = h_d.rearrange("(n p) d -> n p d", p=P)
    c_r = c_d.rearrange("(n p) d -> n p d", p=P)
    cb_r = cb_d.rearrange("(n p) d -> n p d", p=P)
    og_r = og_d.rearrange("(n p) d -> n p d", p=P)
    hn_r = hn_d.rearrange("(n p) d -> n p d", p=P)
    cn_r = cn_d.rearrange("(n p) d -> n p d", p=P)
    cbn_r = cbn_d.rearrange("(n p) d -> n p d", p=P)
    dec_r = dec_d.rearrange("(n p) d -> n p d", p=P)

    def dsl(r, b0, n):  # dram slice of n row-blocks as [P, n, H]
        return r[b0 : b0 + n].rearrange("n p d -> p n d")

    with TileContext(nc) as tc, ExitStack() as ctx:
        pool = lambda name, bufs, **kw: ctx.enter_context(
            tc.tile_pool(name=name, bufs=bufs, **kw)
        )
        const = pool("const", 1)
        hp = pool("hp", 2)
        cp = pool("cp", 2)
        cbp = pool("cbp", 2)
        hTp = pool("hTp", 2)
        spp = pool("spp", 1)
        ep = pool("ep", 2)
        Ap = pool("Ap", 1)
        zp = pool("zp", 1)
        r2p = pool("r2p", 1)
        hhp = pool("hhp", 2)
        ptp = pool("ptp", 1, space="PSUM")
        pdp = pool("pdp", 1, space="PSUM")
        pgp = pool("pgp", 2, space="PSUM")

        wt_sb = const.tile([P, 2, G * H], F32R)
        nc.sync.dma_start(wt_sb[:], wt_d.rearrange("c k g -> k c g"))
        bia_sb = const.tile([1, G * H], BF16)
        nc.sync.dma_start(bia_sb[:], bia_d[:, :])
        ndt_sb = const.tile([P, nblk], F32)
        nc.sync.dma_start(ndt_sb[:], ndt_d[:, :])
        ones_sb = const.tile([1, P], BF16)
        nc.vector.memset(ones_sb[:], 1.0)
        ident = const.tile([P, P], F32)
        masks.make_identity(nc, ident[:])
        tokp = pool("tokp", 2)

        # ACT table-set phase tokens: zero-valued [P,1] tiles whose data deps
        # force all sigmoid-set ops of supergroup g after the exp/ln-set ops
        # of g (tokB), and the exp-set ops of g+1 after sigmoid phase of g
        # (tokA) — keeps table switches at 2 per supergroup.
        tokA = None  # gates u-ops of current supergroup
        r2_last = None  # last r2 tile of previous supergroup (tanh_c output)

        for g in range(nblk // sg):
            # ---------------- phase A: transpose + decay/e chain ----------------
            if r2_last is not None:
                tokA = tokp.tile([P, 1], F32)
                nc.vector.tensor_scalar_mul(tokA[:], r2_last[:, st - 1, H : H + 1], 0.0)
            hT = hTp.tile([P, sg, H], F32R)
            sp = spp.tile([P, sg, H], F32)
            e = ep.tile([P, sg, H], F32)
            for s in range(sg // st):
                b0 = g * sg + s * st
                h_t = hp.tile([P, st, H], F32)
                nc.sync.dma_start(h_t[:], dsl(h_r, b0, st))
                for j in range(st):
                    jj = s * st + j
                    tp = ptp.tile([P, H], F32)
                    nc.tensor.transpose(tp[:, 0:P], h_t[:, j, 0:P], ident[:])
                    nc.tensor.transpose(tp[:, P : 2 * P], h_t[:, j, P : 2 * P], ident[:])
                    nc.scalar.copy(hT[:, jj, :], tp[:])
                    dps = pdp.tile([P, H], F32)
                    nc.tensor.matmul(
                        dps[:], ones_sb[:], bia_sb[:, 6 * H : 7 * H],
                        start=True, stop=False,
                    )
                    for ck in range(2):
                        nc.tensor.matmul(
                            dps[:],
                            hT[:, jj, ck * P : (ck + 1) * P],
                            wt_sb[:, ck, 6 * H : 7 * H],
                            start=False, stop=(ck == 1),
                        )
                    if tokA is not None:  # u = exp(g_d)
                        nc.scalar.activation(sp[:, jj, :], dps[:], AF.Exp, bias=tokA[:])
                    else:
                        nc.scalar.activation(sp[:, jj, :], dps[:], AF.Exp)
            nc.scalar.activation(sp[:], sp[:], AF.Ln, bias=1.0)  # decay = ln(1+u)
            nc.sync.dma_start(dsl(dec_r, g * sg, sg), sp[:])
            for jj in range(sg):
                nc.scalar.activation(
                    e[:, jj, :], sp[:, jj, :], AF.Exp,
                    scale=ndt_sb[:, g * sg + jj : g * sg + jj + 1],
                )
            # ---------------- phase B: gates + elementwise + outputs ----------------
            tokB = tokp.tile([P, 1], F32)
            nc.vector.tensor_scalar_mul(tokB[:], e[:, sg - 1, 0:1], 0.0)
            for s in range(sg // st):
                b0 = g * sg + s * st
                c_t = cp.tile([P, st, H], F32)
                cb_t = cbp.tile([P, st, H], F32)
                nc.sync.dma_start(c_t[:], dsl(c_r, b0, st))
                nc.sync.dma_start(cb_t[:], dsl(cb_r, b0, st))
                A = Ap.tile([P, st, 5 * H], F32)
                z = zp.tile([P, st, H], F32)
                r2 = r2p.tile([P, st, 2 * H], F32)
                hh = hhp.tile([P, st, 2 * H], F32)
                for j in range(st):
                    jj = s * st + j
                    gps = pgp.tile([P, 6 * H], F32)
                    for grp in range(3):
                        lo = grp * 2 * H
                        nc.tensor.matmul(
                            gps[:, lo : lo + 2 * H], ones_sb[:],
                            bia_sb[:, lo : lo + 2 * H], start=True, stop=False,
                        )
                        for ck in range(2):
                            nc.tensor.matmul(
                                gps[:, lo : lo + 2 * H],
                                hT[:, jj, ck * P : (ck + 1) * P],
                                wt_sb[:, ck, lo : lo + 2 * H],
                                start=False, stop=(ck == 1),
                            )
                    nc.scalar.activation(
                        A[:, j, 0 : 5 * H], gps[:, 0 : 5 * H], AF.Sigmoid, bias=tokB[:]
                    )
                    nc.scalar.activation(
                        z[:, j, :], gps[:, 5 * H : 6 * H], AF.Tanh, bias=tokB[:]
                    )
                est = e[:, s * st : (s + 1) * st, :]
                nc.vector.tensor_sub(c_t[:], c_t[:], cb_t[:])
                nc.vector.tensor_mul(c_t[:], c_t[:], est)
                nc.vector.tensor_add(r2[:, :, 0:H], c_t[:], cb_t[:])  # c_after
                nc.scalar.activation(r2[:, :, H : 2 * H], r2[:, :, 0:H], AF.Tanh)
                nc.vector.tensor_mul(hh[:], A[:, :, 0 : 2 * H], r2[:])  # mf | h_new
                nc.vector.tensor_mul(A[:, :, 2 * H : 3 * H], A[:, :, 2 * H : 3 * H], cb_t[:])
                nc.vector.tensor_mul(A[:, :, 3 * H : 4 * H], A[:, :, 3 * H : 4 * H], z[:])
                nc.vector.tensor_mul(A[:, :, 4 * H : 5 * H], A[:, :, 4 * H : 5 * H], z[:])
                nc.sync.dma_start(dsl(og_r, b0, st), A[:, :, H : 2 * H])
                nc.vector.tensor_add(r2[:, :, 0:H], hh[:, :, 0:H], A[:, :, 3 * H : 4 * H])
                nc.vector.tensor_add(
                    r2[:, :, H : 2 * H], A[:, :, 2 * H : 3 * H], A[:, :, 4 * H : 5 * H]
                )
                nc.sync.dma_start(dsl(hn_r, b0, st), hh[:, :, H : 2 * H])
                nc.sync.dma_start(dsl(cn_r, b0, st), r2[:, :, 0:H])
                nc.sync.dma_start(dsl(cbn_r, b0, st), r2[:, :, H : 2 * H])
                r2_last = r2
    nc.compile()
    return nc


def prep_weights(W, b):
    """W [7,256,256] fp32, b [7,256] fp32 -> wt [2,128,1792] f32, bias [1,1792] bf16."""
    import ml_dtypes

    Wp = np.ascontiguousarray(W[GATE_PERM])          # [7,256,256] (g,o,h)
    wt = np.ascontiguousarray(np.transpose(Wp, (2, 0, 1)).reshape(H, G * H))
    wt = np.ascontiguousarray(wt.reshape(2, P, G * H)).astype(np.float32)
    bp = np.ascontiguousarray(b[GATE_PERM]).reshape(1, G * H)
    bia = bp.astype(ml_dtypes.bfloat16)
    return wt, bia


_RUNNER = None


def _make_runner(nc):
    """Cached shard_map-jitted executor for nc across 8 cores.

    Mirrors bass2jax.run_bass_via_pjrt's multi-core branch, but built once so
    repeat kernel() calls reuse the jit cache instead of re-tracing.
    """
    import jax
    from jax.experimental.shard_map import shard_map
    from jax.sharding import Mesh, PartitionSpec

    from concourse import bass2jax, mybir

    bass2jax.install_neuronx_cc_hook()
    p = bass2jax._bass_exec_p

    part_name = nc.partition_id_tensor.name if nc.partition_id_tensor else None
    in_names, out_names, out_avals = [], [], []
    for alloc in nc.m.functions[0].allocations:
        if not isinstance(alloc, mybir.MemoryLocationSet):
            continue
        name = alloc.memorylocations[0].name
        if alloc.kind == "ExternalInput":
            if name != part_name:
                in_names.append(name)
        elif alloc.kind == "ExternalOutput":
            out_names.append(name)
            out_avals.append(
                jax.core.ShapedArray(tuple(alloc.tensor_shape), mybir.dt.np(alloc.dtype))
            )
    n_params = len(in_names)
    all_in = in_names + out_names
    if part_name is not None:
        all_in = all_in + [part_name]

    def _body(*args):
        operands = list(args)
        if part_name is not None:
            operands.append(bass2jax.partition_id_tensor())
        return tuple(
            p.bind(
                *operands,
                out_avals=tuple(out_avals),
                in_names=tuple(all_in),
                out_names=tuple(out_names),
                lowering_input_output_aliases=(),
                sim_require_finite=True,
                sim_require_nnan=True,
                nc=nc,
            )
        )

    devices = jax.devices()[:NCORES]
    mesh = Mesh(np.asarray(devices), ("core",))
    nin = n_params + len(out_names)
    sharded = jax.jit(
        shard_map(
            _body,
            mesh=mesh,
            in_specs=(PartitionSpec("core"),) * nin,
            out_specs=(PartitionSpec("core"),) * len(out_names),
            check_rep=False,
        ),
        donate_argnums=tuple(range(n_params, nin)),
        keep_unused=True,
    )
    return sharded, in_names, out_names, out_avals, mesh


def get_runner():
    global _NC, _RUNNER
    if _RUNNER is None:
        if _NC is None:
            _NC = build_nc()
        _RUNNER = _make_runner(_NC)
    return _RUNNER


def make_concat_inputs(inter_times, h_ti, c_ti, cbar, W, b):
    """Global (8*shape[0], ...) arrays keyed by dram tensor name."""
    inter_times = np.asarray(inter_times, dtype=np.float32)
    wt, bia = prep_weights(np.asarray(W, np.float32), np.asarray(b, np.float32))
    negdt = np.ascontiguousarray(
        -inter_times.reshape(NCORES, NBLK, P).transpose(0, 2, 1)
    ).reshape(NCORES * P, NBLK)
    return {
        "h": np.ascontiguousarray(np.asarray(h_ti, dtype=np.float32)),
        "c": np.ascontiguousarray(np.asarray(c_ti, dtype=np.float32)),
        "cbar": np.ascontiguousarray(np.asarray(cbar, dtype=np.float32)),
        "negdt": negdt,
        "wt": np.ascontiguousarray(np.broadcast_to(wt, (NCORES,) + wt.shape)).reshape(
            NCORES * wt.shape[0], *wt.shape[1:]
        ),
        "bias": np.ascontiguousarray(
            np.broadcast_to(bia, (NCORES,) + bia.shape)
        ).reshape(NCORES * bia.shape[0], *bia.shape[1:]),
    }


def device_zeros(out_avals, mesh):
    import jax.numpy as jnp
    from jax.sharding import NamedSharding, PartitionSpec

    sh = NamedSharding(mesh, PartitionSpec("core"))
    return [
        jnp.zeros((NCORES * a.shape[0], *a.shape[1:]), a.dtype, device=sh)
        for a in out_avals
    ]


def kernel(inter_times, h_ti, c_ti, cbar, W, b):
    sharded, in_names, out_names, out_avals, mesh = get_runner()
    cat = make_concat_inputs(inter_times, h_ti, c_ti, cbar, W, b)
    zeros = device_zeros(out_avals, mesh)
    out_arrs = sharded(*[cat[n] for n in in_names], *zeros)
    by_name = {n: np.asarray(a) for n, a in zip(out_names, out_arrs)}
    return tuple(by_name[n] for n in ["og", "hn", "cn", "cbn", "dec"])
